# revision 1
# baseline (speedup 1.0000x reference)
import sys

sys.path.insert(0, "/opt/trn_rl_repo")

import numpy as np

NCORES = 8
B, FULL_N, D = 4, 2048, 1024
NH = 16
DK = 64  # head dim
HPC = NH // NCORES  # heads per core = 2
CW = HPC * DK  # output columns per core = 128
DC = D // 128  # D chunks = 8

_CACHE = {}
LAST_RESULTS = None


def _build(n_rows):
    """SPMD Bass program for one core. Each core computes batch-0 attention
    for its 2 heads (the reference only uses att[0]) and adds it to its
    column slice of tgt for all batches.

    tgt[0]/memory[0] arrive host-transposed ([D, N]) and pre-rounded to the
    fp32r grid, declared float32r, so DMA lands matmul-ready (fp32r runs at
    full PE rate for moving dim 512). Scores are computed transposed (k on
    partitions) so softmax's P feeds P.T@V with no P transposes; V carries
    an appended ones column so the same accumulation yields softmax row
    sums. K/Q/V live in per-512-row-group tiles so attention chunks can
    start as soon as their group's projections finish."""
    import concourse.mybir as mybir
    import concourse.tile as tile
    from concourse import bacc
    from concourse.masks import make_identity

    fp32 = mybir.dt.float32
    fp32r = mybir.dt.float32r
    bf16 = mybir.dt.bfloat16

    RT = n_rows // 128  # row tiles
    G = n_rows // 512  # 512-row groups
    QG = G
    KC = RT  # k chunks of 128

    nc = bacc.Bacc(None, target_bir_lowering=False)
    tgt0t = nc.declare_dram_parameter("tgt0t", [D, n_rows], fp32r, isOutput=False)
    mem0t = nc.declare_dram_parameter("mem0t", [D, n_rows], fp32r, isOutput=False)
    wqt = nc.declare_dram_parameter("wqt", [D, CW], fp32r, isOutput=False)
    wkt = nc.declare_dram_parameter("wkt", [D, CW], fp32r, isOutput=False)
    wvt = nc.declare_dram_parameter("wvt", [D, CW], fp32r, isOutput=False)
    tgtc = nc.declare_dram_parameter("tgtc", [B, n_rows, CW], fp32, isOutput=False)
    outc = nc.declare_dram_parameter("outc", [B, n_rows, CW], fp32, isOutput=True)

    Exp = mybir.ActivationFunctionType.Exp
    scale = 1.0 / np.sqrt(DK)

    with tile.TileContext(nc) as tc:
        with (
            tc.tile_pool(name="const", bufs=1) as const,
            tc.tile_pool(name="persist", bufs=1) as persist,
        ):
            ident = const.tile([128, 128], fp32)
            make_identity(nc, ident)

            # per-group K/Q/V tiles (fine-grained deps -> phase overlap)
            KT_gs = [
                persist.tile([128, 512], fp32r, tag=f"KT{g}", name=f"KT{g}")
                for g in range(G)
            ]
            QT_gs = [
                persist.tile([128, 512], fp32r, tag=f"QT{g}", name=f"QT{g}")
                for g in range(G)
            ]
            Vp_gs = [
                persist.tile([128, HPC, 4, DK + 1], bf16, tag=f"Vp{g}", name=f"Vp{g}")
                for g in range(G)
            ]
            att_sb = persist.tile([128, RT, CW], fp32, tag="att")
            tgtc_sb = persist.tile([128, B, RT, CW], fp32, tag="tgtc")

            ones_f32 = const.tile([128, HPC, 4], fp32, tag="ones")
            nc.vector.memset(ones_f32, 1.0)

            # ---- Phase A: loads + QKV projections (per 512-row group) ----
            with (
                tc.tile_pool(name="wst", bufs=1) as wst_pool,
                tc.tile_pool(name="grp", bufs=2) as grp_pool,
                tc.tile_pool(name="vtg", bufs=2) as vt_pool,
                tc.tile_pool(name="ps_w", bufs=1, space="PSUM") as ps_w,
                tc.tile_pool(name="ps_acc", bufs=2, space="PSUM") as ps_acc,
            ):
                # PE warmup during the initial DMA wait (HAM un-throttle)
                for _ in range(16):
                    pw = ps_w.tile([128, 128], fp32, tag="warm")
                    nc.tensor.transpose(pw, ident, ident)

                WTs = {}
                for name, w in (("q", wqt), ("k", wkt), ("v", wvt)):
                    wt = wst_pool.tile([128, DC, CW], fp32r, tag=f"wt{name}")
                    nc.sync.dma_start(
                        out=wt, in_=w[:, :].rearrange("(c p) q -> p c q", p=128)
                    )
                    WTs[name] = wt

                def emit_mem_group(g):
                    memT_g = grp_pool.tile(
                        [128, DC, 512], fp32r, tag="memTg", name=f"memT{g}"
                    )
                    for d in range(DC):
                        nc.sync.dma_start(
                            out=memT_g[:, d, :],
                            in_=mem0t[
                                d * 128 : (d + 1) * 128, g * 512 : (g + 1) * 512
                            ],
                        )
                    pk = ps_acc.tile([128, 512], fp32, tag="acc")
                    for d in range(DC):
                        nc.tensor.matmul(
                            pk, WTs["k"][:, d, :], memT_g[:, d, :],
                            start=(d == 0), stop=(d == DC - 1),
                        )
                    nc.vector.tensor_copy(out=KT_gs[g], in_=pk)
                    pv = ps_acc.tile([128, 512], fp32, tag="acc")
                    for d in range(DC):
                        nc.tensor.matmul(
                            pv, WTs["v"][:, d, :], memT_g[:, d, :],
                            start=(d == 0), stop=(d == DC - 1),
                        )
                    vt_g = vt_pool.tile([128, 512], fp32, tag="vtg")
                    nc.vector.tensor_copy(out=vt_g, in_=pv)
                    for t in range(4):
                        ptr = ps_w.tile([128, 128], fp32, tag="warm")
                        nc.tensor.transpose(ptr, vt_g[:, t * 128 : (t + 1) * 128], ident)
                        nc.vector.tensor_copy(
                            out=Vp_gs[g][:, 0, t, 0:DK], in_=ptr[:, 0:DK]
                        )
                        nc.vector.tensor_copy(
                            out=Vp_gs[g][:, 1, t, 0:DK], in_=ptr[:, DK : 2 * DK]
                        )
                    nc.vector.tensor_copy(out=Vp_gs[g][:, :, :, DK], in_=ones_f32)

                def emit_tgt_group(g):
                    tgtT_g = grp_pool.tile(
                        [128, DC, 512], fp32r, tag="tgtTg", name=f"tgtT{g}"
                    )
                    for d in range(DC):
                        nc.sync.dma_start(
                            out=tgtT_g[:, d, :],
                            in_=tgt0t[
                                d * 128 : (d + 1) * 128, g * 512 : (g + 1) * 512
                            ],
                        )
                    pq = ps_acc.tile([128, 512], fp32, tag="acc")
                    for d in range(DC):
                        nc.tensor.matmul(
                            pq, WTs["q"][:, d, :], tgtT_g[:, d, :],
                            start=(d == 0), stop=(d == DC - 1),
                        )
                    nc.vector.tensor_copy(out=QT_gs[g], in_=pq)


                # ---- Phase B: attention per q-group, heads paired ----
                with (
                    tc.tile_pool(name="pt", bufs=1) as pt_pool,
                    tc.tile_pool(name="usb", bufs=2) as usb_pool,
                    tc.tile_pool(name="small", bufs=8) as small_pool,
                    tc.tile_pool(name="ps_st", bufs=2, space="PSUM") as ps_st,
                    tc.tile_pool(name="ps_u", bufs=1, space="PSUM") as ps_u,
                ):
                    def emit_st_block(qg, pts, jp):
                        # one exp pair: chunks 2*jp, 2*jp+1 for both heads
                        psts = [
                            ps_st.tile(
                                [128, 2, 512], fp32, tag="st", name=f"st{qg}_{jp}_{h}"
                            )
                            for h in range(HPC)
                        ]
                        for jj in range(2):
                            j = jp * 2 + jj
                            kg, kt = j // 4, j % 4
                            for h in range(HPC):
                                hs = h * DK
                                nc.tensor.matmul(
                                    psts[h][:, jj, :],
                                    KT_gs[kg][hs : hs + DK, kt * 128 : (kt + 1) * 128],
                                    QT_gs[qg][hs : hs + DK, :],
                                    start=True, stop=True,
                                )
                        for h in range(HPC):
                            nc.scalar.activation(
                                out=pts[h][:, jp * 2 : jp * 2 + 2, :],
                                in_=psts[h],
                                func=Exp,
                                scale=float(scale),
                            )

                    all_pts = {}
                    # ramp: stream qg0's score chunks between memory groups so
                    # ScalarE starts exp work as early as possible
                    emit_tgt_group(0)
                    all_pts[0] = [
                        pt_pool.tile(
                            [128, KC, 512], bf16, tag=f"pt{h}", name=f"pt{h}_0"
                        )
                        for h in range(HPC)
                    ]
                    for g in range(G):
                        emit_mem_group(g)
                        for jp in range(2 * g, 2 * g + 2):
                            emit_st_block(0, all_pts[0], jp)

                    for b in range(B):
                        nc.sync.dma_start(
                            out=tgtc_sb[:, b, :, :],
                            in_=tgtc[b, :, :].rearrange("(t p) c -> p t c", p=128),
                        )

                    for qg in range(QG):
                        if qg + 1 < QG:
                            emit_tgt_group(qg + 1)
                            all_pts[qg + 1] = [
                                pt_pool.tile(
                                    [128, KC, 512], bf16, tag=f"pt{h}",
                                    name=f"pt{h}_{qg+1}",
                                )
                                for h in range(HPC)
                            ]
                        qsl = slice(qg * 512, (qg + 1) * 512)
                        pts = all_pts[qg]
                        if qg > 0:
                            for jp in range(KC // 2):
                                emit_st_block(qg, pts, jp)
                        for h in range(HPC):
                            hs = h * DK
                            pu = ps_u.tile([DK + 1, 512], fp32, tag="u")
                            for j in range(KC):
                                nc.tensor.matmul(
                                    pu,
                                    Vp_gs[j // 4][:, h, j % 4, :],
                                    pts[h][:, j, :],
                                    start=(j == 0), stop=(j == KC - 1),
                                )
                            pu_sb = usb_pool.tile([DK + 1, 512], fp32, tag="usb")
                            nc.vector.tensor_copy(out=pu_sb, in_=pu)
                            for s in range(4):
                                pat = ps_w.tile([128, 128], fp32, tag="warm")
                                nc.tensor.transpose(
                                    pat[:, 0 : DK + 1],
                                    pu_sb[:, s * 128 : (s + 1) * 128],
                                    ident[0 : DK + 1, 0 : DK + 1],
                                )
                                rec = small_pool.tile([128, 1], fp32, tag="rec")
                                nc.vector.reciprocal(rec, pat[:, DK : DK + 1])
                                nc.vector.tensor_scalar_mul(
                                    att_sb[:, qg * 4 + s, hs : hs + DK],
                                    in0=pat[:, 0:DK],
                                    scalar1=rec,
                                )
                        # final broadcast add + store for this q-group's rows
                        for b in range(B):
                            nc.vector.tensor_add(
                                out=tgtc_sb[:, b, qg * 4 : (qg + 1) * 4, :],
                                in0=tgtc_sb[:, b, qg * 4 : (qg + 1) * 4, :],
                                in1=att_sb[:, qg * 4 : (qg + 1) * 4, :],
                            )
                            nc.sync.dma_start(
                                out=outc[b, qsl, :].rearrange(
                                    "(t p) c -> p t c", p=128
                                ),
                                in_=tgtc_sb[:, b, qg * 4 : (qg + 1) * 4, :],
                            )

    nc.finalize()
    return nc


def _get_nc(n_rows):
    if n_rows not in _CACHE:
        _CACHE[n_rows] = _build(n_rows)
    return _CACHE[n_rows]


def _round_fp32r(x):
    """Round fp32 to the fp32r grid (11 explicit mantissa bits, RNE)."""
    v = np.ascontiguousarray(x, dtype=np.float32).view(np.uint32)
    lo = v & np.uint32(0xFFF)
    base = v & ~np.uint32(0xFFF)
    lsb = (v >> np.uint32(12)) & np.uint32(1)
    up = (lo > 0x800) | ((lo == 0x800) & (lsb == 1))
    out = base + (up.astype(np.uint32) << np.uint32(12))
    return out.view(np.float32)


def _run(tgt, memory, Wq, Wk, Wv, trace=False):
    global LAST_RESULTS
    from concourse.bass_utils import run_bass_kernel_spmd

    n_rows = tgt.shape[1]
    nc = _get_nc(n_rows)

    tgt = np.ascontiguousarray(tgt, dtype=np.float32)
    memory = np.ascontiguousarray(memory, dtype=np.float32)
    tgt0t = _round_fp32r(np.ascontiguousarray(tgt[0].T))
    mem0t = _round_fp32r(np.ascontiguousarray(memory[0].T))

    in_maps = []
    for c in range(NCORES):
        sl = slice(c * CW, (c + 1) * CW)
        in_maps.append(
            {
                "tgt0t": tgt0t,
                "mem0t": mem0t,
                "wqt": _round_fp32r(Wq[sl, :].T),
                "wkt": _round_fp32r(Wk[sl, :].T),
                "wvt": _round_fp32r(Wv[sl, :].T),
                "tgtc": np.ascontiguousarray(tgt[:, :, sl]),
            }
        )
    res = run_bass_kernel_spmd(nc, in_maps, list(range(NCORES)), trace=trace)
    LAST_RESULTS = res
    out = np.concatenate([res.results[c]["outc"] for c in range(NCORES)], axis=2)
    return out


def kernel(tgt, memory, Wq, Wk, Wv):
    return _run(tgt, memory, Wq, Wk, Wv)



# revision 6
# speedup vs baseline: 1.3398x; 1.3398x over previous
import sys

sys.path.insert(0, "/opt/trn_rl_repo")

import numpy as np

NCORES = 8
B, FULL_N, D = 4, 2048, 1024
NH = 16
DK = 64  # head dim
HPC = NH // NCORES  # heads per core = 2
CW = HPC * DK  # output columns per core = 128
DC = D // 128  # D chunks = 8
VW = 80  # padded V width per head (64 dims + ones col + pad to %16)

# exp routing: key chunks (j % 16) in this set are exponentiated on DVE via
# the Schraudolph bit-hack (fp32 -> saturating uint8 == fp8e4m3 bit pattern);
# the rest go through ScalarE's exact Exp with fp8e4 output.
DVE_CHUNKS = frozenset({2, 6, 10, 13})
# byte = psum_score * (8/ln2)/8 + (56 - 24 - sigma)  [fp8e4m3, y=exp(s)/8]
A_HACK = float(1.0 / np.log(2.0))  # 1.4427 (includes the 1/sqrt(dk)=1/8 fold)
B_HACK = 31.537  # 32 - 0.463 Schraudolph mantissa-balance

_CACHE = {}
LAST_RESULTS = None


def _build(n_rows):
    """SPMD Bass program for one core. Each core computes batch-0 attention
    for its 2 heads (the reference only uses att[0]) and adds it to its
    column slice of tgt for all batches.

    All HBM traffic is fp16 (host-converted). Score matmuls use 64-contraction
    row tiling so both heads' chunks run concurrently on the PE. Softmax
    exponentials are split between ScalarE (exact Exp -> fp8e4, scaled by
    2^-3) and DVE (saturating affine-to-uint8 bit hack). P@V runs in fp8
    DoubleRow mode (256-key contraction per pass) with a ones column at
    position 64 of each 80-wide V block yielding softmax row sums for free."""
    import concourse.mybir as mybir
    import concourse.tile as tile
    from concourse import bacc
    from concourse.masks import make_identity

    fp32 = mybir.dt.float32
    fp16 = mybir.dt.float16
    fp8 = mybir.dt.float8e4
    u8 = mybir.dt.uint8

    RT = n_rows // 128  # row tiles = 16
    G = n_rows // 512  # 512-row groups = 4
    QG = G
    KC = RT  # key chunks of 128
    JP = KC // 2  # key chunk pairs = 8

    nc = bacc.Bacc(None, target_bir_lowering=False)
    tgt0t = nc.declare_dram_parameter("tgt0t", [D, n_rows], fp16, isOutput=False)
    mem0t = nc.declare_dram_parameter("mem0t", [D, n_rows], fp16, isOutput=False)
    wqt = nc.declare_dram_parameter("wqt", [D, CW], fp16, isOutput=False)
    wkt = nc.declare_dram_parameter("wkt", [D, CW], fp16, isOutput=False)
    wvt = nc.declare_dram_parameter("wvt", [D, CW], fp16, isOutput=False)
    tgtc = nc.declare_dram_parameter("tgtc", [B, n_rows, CW], fp16, isOutput=False)
    outc = nc.declare_dram_parameter("outc", [B, n_rows, CW], fp16, isOutput=True)

    Exp = mybir.ActivationFunctionType.Exp
    DR = mybir.MatmulPerfMode.DoubleRow
    mult = mybir.AluOpType.mult
    add = mybir.AluOpType.add

    with tile.TileContext(nc) as tc:
        with (
            tc.tile_pool(name="const", bufs=1) as const,
            tc.tile_pool(name="persist", bufs=1) as persist,
        ):
            ident16 = const.tile([128, 128], fp16)
            make_identity(nc, ident16)
            bias_exp = const.tile([128, 1], fp32, tag="bias")
            nc.vector.memset(bias_exp, -3.0 * float(np.log(2.0)))

            KT_gs = [
                persist.tile([128, 512], fp16, tag=f"KT{g}", name=f"KT{g}")
                for g in range(G)
            ]
            QT_gs = [
                persist.tile([128, 512], fp16, tag=f"QT{g}", name=f"QT{g}")
                for g in range(G)
            ]
            # [keys, pair jp, pair half i, h*VW + dim]; ones at h*VW+64
            Vp = persist.tile([128, JP, 2, HPC * VW], fp8, tag="Vp")
            nc.vector.memset(Vp, 0.0)
            Vp_h = Vp.rearrange("p jp i (h c) -> p jp i h c", h=HPC)
            nc.vector.memset(Vp_h[:, :, :, :, DK : DK + 1], 1.0)

            att_sb = persist.tile([128, RT, CW], fp16, tag="att")
            tgtc_sb = persist.tile([128, B, RT, CW], fp16, tag="tgtc")

            with (
                tc.tile_pool(name="wst", bufs=1) as wst_pool,
                tc.tile_pool(name="grp", bufs=2) as grp_pool,
                tc.tile_pool(name="vtg", bufs=2) as vt_pool,
                tc.tile_pool(name="usb", bufs=2) as usb_pool,
                tc.tile_pool(name="small", bufs=8) as small_pool,
                tc.tile_pool(name="pt", bufs=2) as pt_pool,
                tc.tile_pool(name="ps_acc", bufs=2, space="PSUM") as ps_acc,
                tc.tile_pool(name="ps_w", bufs=1, space="PSUM") as ps_w,
                tc.tile_pool(name="ps_st", bufs=2, space="PSUM") as ps_st,
                tc.tile_pool(name="ps_u", bufs=1, space="PSUM") as ps_u,
            ):
                # PE warmup during the initial DMA wait (HAM un-throttle);
                # real matmuls (transpose mode doesn't count as PE-busy).
                def emit_warm(n):
                    for _ in range(n):
                        pw = ps_acc.tile([128, 512], fp32, tag="acc")
                        nc.tensor.matmul(
                            pw[:, 0:128], ident16, ident16, start=True, stop=True
                        )

                emit_warm(48)

                WTs = {}
                for name, w in (("q", wqt), ("k", wkt), ("v", wvt)):
                    wt = wst_pool.tile([128, DC, CW], fp16, tag=f"wt{name}")
                    nc.sync.dma_start(
                        out=wt, in_=w[:, :].rearrange("(c p) q -> p c q", p=128)
                    )
                    WTs[name] = wt

                def emit_tgt_group(g):
                    tgtT_g = grp_pool.tile(
                        [128, DC, 512], fp16, tag="tgtTg", name=f"tgtT{g}"
                    )
                    for d in range(DC):
                        nc.sync.dma_start(
                            out=tgtT_g[:, d, :],
                            in_=tgt0t[
                                d * 128 : (d + 1) * 128, g * 512 : (g + 1) * 512
                            ],
                        )
                    pq = ps_acc.tile([128, 512], fp32, tag="acc")
                    for d in range(DC):
                        nc.tensor.matmul(
                            pq, WTs["q"][:, d, :], tgtT_g[:, d, :],
                            start=(d == 0), stop=(d == DC - 1),
                        )
                    nc.vector.tensor_copy(out=QT_gs[g], in_=pq)

                def emit_mem_group(g):
                    memT_g = grp_pool.tile(
                        [128, DC, 512], fp16, tag="memTg", name=f"memT{g}"
                    )
                    for d in range(DC):
                        nc.sync.dma_start(
                            out=memT_g[:, d, :],
                            in_=mem0t[
                                d * 128 : (d + 1) * 128, g * 512 : (g + 1) * 512
                            ],
                        )
                    pk = ps_acc.tile([128, 512], fp32, tag="acc")
                    for d in range(DC):
                        nc.tensor.matmul(
                            pk, WTs["k"][:, d, :], memT_g[:, d, :],
                            start=(d == 0), stop=(d == DC - 1),
                        )
                    nc.vector.tensor_copy(out=KT_gs[g], in_=pk)
                    pv = ps_acc.tile([128, 512], fp32, tag="acc")
                    for d in range(DC):
                        nc.tensor.matmul(
                            pv, WTs["v"][:, d, :], memT_g[:, d, :],
                            start=(d == 0), stop=(d == DC - 1),
                        )
                    vt_g = vt_pool.tile([128, 512], fp16, tag="vtg")
                    nc.vector.tensor_copy(out=vt_g, in_=pv)
                    for t in range(4):
                        j = 4 * g + t
                        jp, i = j // 2, j % 2
                        tw = ps_w.tile([128, 128], fp16, tag="tw")
                        nc.tensor.transpose(
                            tw, vt_g[:, t * 128 : (t + 1) * 128], ident16
                        )
                        nc.vector.tensor_copy(
                            out=Vp_h[:, jp, i, :, 0:DK],
                            in_=tw.rearrange("p (h c) -> p h c", h=HPC),
                        )

                def emit_score_chunk(qg, j, pts, warm=False):
                    g, t = j // 4, j % 4
                    st = ps_st.tile(
                        [128, HPC, 512], fp32, tag="st", name=f"st{qg}_{j}"
                    )
                    for h in range(HPC):
                        hs = h * DK
                        nc.tensor.matmul(
                            st[:, h, :],
                            KT_gs[g][hs : hs + DK, t * 128 : (t + 1) * 128],
                            QT_gs[qg][hs : hs + DK, :],
                            start=True, stop=True,
                        )
                    if warm:
                        emit_warm(1)
                    jp, i = j // 2, j % 2
                    dst = pts[:, jp, i, :, :]  # [128, h, 512] fp8
                    if j in DVE_CHUNKS:
                        nc.vector.tensor_scalar(
                            out=dst.bitcast(u8), in0=st,
                            scalar1=A_HACK, scalar2=B_HACK,
                            op0=mult, op1=add,
                        )
                    else:
                        nc.scalar.activation(
                            out=dst, in_=st, func=Exp, scale=0.125, bias=bias_exp
                        )

                def emit_pv(qg, h, pts):
                    pu = ps_u.tile([VW, 512], fp32, tag="u", name=f"u{qg}_{h}")
                    for jp in range(JP):
                        nc.tensor.matmul(
                            pu,
                            Vp[:, jp, :, h * VW : (h + 1) * VW],
                            pts[:, jp, :, h, :],
                            start=(jp == 0), stop=(jp == JP - 1),
                            perf_mode=DR,
                        )
                    pu_sb = usb_pool.tile([VW, 512], fp16, tag="usb")
                    nc.vector.tensor_copy(out=pu_sb, in_=pu)
                    hs = h * DK
                    for s in range(4):
                        ta = ps_w.tile([128, 128], fp16, tag="tw")
                        nc.tensor.transpose(
                            ta[:, 0:VW],
                            pu_sb[:, s * 128 : (s + 1) * 128],
                            ident16[0:VW, 0:VW],
                        )
                        rec = small_pool.tile([128, 1], fp32, tag="rec")
                        nc.vector.reciprocal(rec, ta[:, DK : DK + 1])
                        nc.vector.tensor_scalar_mul(
                            att_sb[:, qg * 4 + s, hs : hs + DK],
                            in0=ta[:, 0:DK],
                            scalar1=rec,
                        )

                # ---- Phase A: loads, projections, qg0 scores ----
                emit_tgt_group(0)
                pts_tiles = {}
                pts_tiles[0] = pt_pool.tile(
                    [128, JP, 2, HPC, 512], fp8, tag="pts", name="pts0"
                )
                for g in range(G):
                    emit_mem_group(g)
                    for j in range(4 * g, 4 * g + 4):
                        emit_score_chunk(0, j, pts_tiles[0])

                for b in range(B):
                    nc.sync.dma_start(
                        out=tgtc_sb[:, b, :, :],
                        in_=tgtc[b, :, :].rearrange("(t p) c -> p t c", p=128),
                    )

                # ---- Phase B: per q-group scores/exp -> PV -> finalize ----
                for qg in range(QG):
                    if qg + 1 < QG:
                        emit_tgt_group(qg + 1)
                        pts_tiles[qg + 1] = pt_pool.tile(
                            [128, JP, 2, HPC, 512], fp8, tag="pts",
                            name=f"pts{qg + 1}",
                        )
                        for j in range(KC):
                            emit_score_chunk(
                                qg + 1, j, pts_tiles[qg + 1], warm=(j % 4 == 2)
                            )
                    for h in range(HPC):
                        emit_pv(qg, h, pts_tiles[qg])
                    qsl = slice(qg * 512, (qg + 1) * 512)
                    for b in range(B):
                        nc.gpsimd.tensor_add(
                            out=tgtc_sb[:, b, qg * 4 : (qg + 1) * 4, :],
                            in0=tgtc_sb[:, b, qg * 4 : (qg + 1) * 4, :],
                            in1=att_sb[:, qg * 4 : (qg + 1) * 4, :],
                        )
                        nc.sync.dma_start(
                            out=outc[b, qsl, :].rearrange(
                                "(t p) c -> p t c", p=128
                            ),
                            in_=tgtc_sb[:, b, qg * 4 : (qg + 1) * 4, :],
                        )

    nc.finalize()
    return nc


def _get_nc(n_rows):
    if n_rows not in _CACHE:
        _CACHE[n_rows] = _build(n_rows)
    return _CACHE[n_rows]


def _run(tgt, memory, Wq, Wk, Wv, trace=False):
    global LAST_RESULTS
    from concourse.bass_utils import run_bass_kernel_spmd

    n_rows = tgt.shape[1]
    nc = _get_nc(n_rows)

    tgt = np.ascontiguousarray(tgt, dtype=np.float32)
    memory = np.ascontiguousarray(memory, dtype=np.float32)
    tgt0t = np.ascontiguousarray(tgt[0].T).astype(np.float16)
    mem0t = np.ascontiguousarray(memory[0].T).astype(np.float16)

    in_maps = []
    for c in range(NCORES):
        sl = slice(c * CW, (c + 1) * CW)
        in_maps.append(
            {
                "tgt0t": tgt0t,
                "mem0t": mem0t,
                "wqt": np.ascontiguousarray(Wq[sl, :].T).astype(np.float16),
                "wkt": np.ascontiguousarray(Wk[sl, :].T).astype(np.float16),
                "wvt": np.ascontiguousarray(Wv[sl, :].T).astype(np.float16),
                "tgtc": np.ascontiguousarray(tgt[:, :, sl]).astype(np.float16),
            }
        )
    res = run_bass_kernel_spmd(nc, in_maps, list(range(NCORES)), trace=trace)
    LAST_RESULTS = res
    out = np.concatenate(
        [res.results[c]["outc"].astype(np.float32) for c in range(NCORES)], axis=2
    )
    return out


def kernel(tgt, memory, Wq, Wk, Wv):
    return _run(tgt, memory, Wq, Wk, Wv)


# revision 12
# speedup vs baseline: 1.4492x; 1.0817x over previous
import sys

sys.path.insert(0, "/opt/trn_rl_repo")

import numpy as np

NCORES = 8
B, FULL_N, D = 4, 2048, 1024
NH = 16
DK = 64  # head dim
HPC = NH // NCORES  # heads per core = 2
CW = HPC * DK  # output columns per core = 128
DC = D // 128  # D chunks = 8
VW = 80  # padded V width per head (64 dims + ones col + pad to %16)

# exp routing: key chunks (j % 16) in this set are exponentiated on DVE via
# the Schraudolph bit-hack (fp32 -> saturating uint8 == fp8e4m3 bit pattern);
# the rest go through ScalarE's exact Exp with fp8e4 output.
DVE_CHUNKS = frozenset({2, 6, 10, 13})
# byte = psum_score * (8/ln2)/8 + (56 - 24 - sigma)  [fp8e4m3, y=exp(s)/8]
A_HACK = float(1.0 / np.log(2.0))  # 1.4427 (includes the 1/sqrt(dk)=1/8 fold)
B_HACK = 31.537  # 32 - 0.463 Schraudolph mantissa-balance

_CACHE = {}
LAST_RESULTS = None


def _build(n_rows):
    """SPMD Bass program for one core. Each core computes batch-0 attention
    for its 2 heads (the reference only uses att[0]) and adds it to its
    column slice of tgt for all batches.

    All HBM traffic is fp16 (host-converted). Score matmuls use 64-contraction
    row tiling so both heads' chunks run concurrently on the PE. Softmax
    exponentials are split between ScalarE (exact Exp -> fp8e4, scaled by
    2^-3) and DVE (saturating affine-to-uint8 bit hack). P@V runs in fp8
    DoubleRow mode (256-key contraction per pass) with a ones column at
    position 64 of each 80-wide V block yielding softmax row sums for free."""
    import concourse.mybir as mybir
    import concourse.tile as tile
    from concourse import bacc
    from concourse.masks import make_identity

    fp32 = mybir.dt.float32
    fp16 = mybir.dt.float16
    fp8 = mybir.dt.float8e4
    u8 = mybir.dt.uint8

    RT = n_rows // 128  # row tiles = 16
    G = n_rows // 512  # 512-row groups = 4
    QG = G
    KC = RT  # key chunks of 128
    JP = KC // 2  # key chunk pairs = 8

    nc = bacc.Bacc(None, target_bir_lowering=False)
    tgt0t = nc.declare_dram_parameter("tgt0t", [D, n_rows], fp16, isOutput=False)
    mem0t = nc.declare_dram_parameter("mem0t", [D, n_rows], fp16, isOutput=False)
    wqt = nc.declare_dram_parameter("wqt", [D, CW], fp16, isOutput=False)
    wkt = nc.declare_dram_parameter("wkt", [D, CW], fp16, isOutput=False)
    wvt = nc.declare_dram_parameter("wvt", [D, CW], fp16, isOutput=False)
    tgtc = nc.declare_dram_parameter("tgtc", [B, n_rows, CW], fp16, isOutput=False)
    outc = nc.declare_dram_parameter("outc", [B, n_rows, CW], fp16, isOutput=True)

    Exp = mybir.ActivationFunctionType.Exp
    DR = mybir.MatmulPerfMode.DoubleRow
    mult = mybir.AluOpType.mult
    add = mybir.AluOpType.add

    with tile.TileContext(nc) as tc:
        with (
            tc.tile_pool(name="const", bufs=1) as const,
            tc.tile_pool(name="persist", bufs=1) as persist,
        ):
            ident16 = const.tile([128, 128], fp16)
            make_identity(nc, ident16)
            bias_exp = const.tile([128, 1], fp32, tag="bias")
            nc.vector.memset(bias_exp, -3.0 * float(np.log(2.0)))

            KT_gs = [
                persist.tile([128, 512], fp16, tag=f"KT{g}", name=f"KT{g}")
                for g in range(G)
            ]
            QT_gs = [
                persist.tile([128, 512], fp16, tag=f"QT{g}", name=f"QT{g}")
                for g in range(G)
            ]
            # [keys, pair jp, pair half i, h*VW + dim]; ones at h*VW+64
            Vp = persist.tile([128, JP, 2, HPC * VW], fp8, tag="Vp")
            nc.vector.memset(Vp, 0.0)
            Vp_h = Vp.rearrange("p jp i (h c) -> p jp i h c", h=HPC)
            nc.vector.memset(Vp_h[:, :, :, :, DK : DK + 1], 1.0)

            att_sb = persist.tile([128, RT, CW], fp16, tag="att")
            tgtc_sb = persist.tile([128, B, RT, CW], fp16, tag="tgtc")

            with (
                tc.tile_pool(name="wst", bufs=1) as wst_pool,
                tc.tile_pool(name="grp", bufs=2) as grp_pool,
                tc.tile_pool(name="vtg", bufs=2) as vt_pool,
                tc.tile_pool(name="usb", bufs=2) as usb_pool,
                tc.tile_pool(name="small", bufs=8) as small_pool,
                tc.tile_pool(name="pt", bufs=2) as pt_pool,
                tc.tile_pool(name="ps_acc", bufs=2, space="PSUM") as ps_acc,
                tc.tile_pool(name="ps_w", bufs=1, space="PSUM") as ps_w,
                tc.tile_pool(name="ps_st", bufs=2, space="PSUM") as ps_st,
                tc.tile_pool(name="ps_u", bufs=1, space="PSUM") as ps_u,
            ):
                # PE warmup during the initial DMA wait (HAM un-throttle);
                # real matmuls (transpose mode doesn't count as PE-busy).
                def emit_warm(n):
                    pw = ps_acc.tile([128, 512], fp32, tag="acc")
                    for _ in range(n):
                        nc.tensor.matmul(
                            pw[:, 0:128], ident16, ident16,
                            start=True, stop=True, skip_group_check=True,
                        )

                emit_warm(40)

                WTs = {}
                for name, w in (("q", wqt), ("k", wkt), ("v", wvt)):
                    wt = wst_pool.tile([128, DC, CW], fp16, tag=f"wt{name}")
                    nc.sync.dma_start(
                        out=wt, in_=w[:, :].rearrange("(c p) q -> p c q", p=128)
                    )
                    WTs[name] = wt

                def emit_tgt_group(g):
                    tgtT_g = grp_pool.tile(
                        [128, DC, 512], fp16, tag="tgtTg", name=f"tgtT{g}"
                    )
                    nc.sync.dma_start(
                        out=tgtT_g,
                        in_=tgt0t[:, g * 512 : (g + 1) * 512].rearrange(
                            "(c p) n -> p c n", p=128
                        ),
                    )
                    pq = ps_acc.tile([128, 512], fp32, tag="acc")
                    for d in range(DC):
                        nc.tensor.matmul(
                            pq, WTs["q"][:, d, :], tgtT_g[:, d, :],
                            start=(d == 0), stop=(d == DC - 1),
                        )
                    nc.vector.tensor_copy(out=QT_gs[g], in_=pq)

                def emit_mem_group(g):
                    memT_g = grp_pool.tile(
                        [128, DC, 512], fp16, tag="memTg", name=f"memT{g}"
                    )
                    nc.sync.dma_start(
                        out=memT_g,
                        in_=mem0t[:, g * 512 : (g + 1) * 512].rearrange(
                            "(c p) n -> p c n", p=128
                        ),
                    )
                    pk = ps_acc.tile([128, 512], fp32, tag="acc")
                    for d in range(DC):
                        nc.tensor.matmul(
                            pk, WTs["k"][:, d, :], memT_g[:, d, :],
                            start=(d == 0), stop=(d == DC - 1),
                        )
                    nc.vector.tensor_copy(out=KT_gs[g], in_=pk)
                    pv = ps_acc.tile([128, 512], fp32, tag="acc")
                    for d in range(DC):
                        nc.tensor.matmul(
                            pv, WTs["v"][:, d, :], memT_g[:, d, :],
                            start=(d == 0), stop=(d == DC - 1),
                        )
                    vt_g = vt_pool.tile([128, 512], fp16, tag="vtg")
                    nc.vector.tensor_copy(out=vt_g, in_=pv)
                    for t in range(4):
                        j = 4 * g + t
                        jp, i = j // 2, j % 2
                        tw = ps_w.tile([128, 128], fp16, tag="tw")
                        nc.tensor.transpose(
                            tw, vt_g[:, t * 128 : (t + 1) * 128], ident16
                        )
                        nc.vector.tensor_copy(
                            out=Vp_h[:, jp, i, :, 0:DK],
                            in_=tw.rearrange("p (h c) -> p h c", h=HPC),
                        )

                def emit_score_chunk(qg, j, pts, warm=False):
                    g, t = j // 4, j % 4
                    st = ps_st.tile(
                        [128, HPC, 512], fp32, tag="st", name=f"st{qg}_{j}"
                    )
                    for h in range(HPC):
                        hs = h * DK
                        nc.tensor.matmul(
                            st[:, h, :],
                            KT_gs[g][hs : hs + DK, t * 128 : (t + 1) * 128],
                            QT_gs[qg][hs : hs + DK, :],
                            start=True, stop=True,
                        )
                    if warm:
                        emit_warm(1)
                    jp, i = j // 2, j % 2
                    dst = pts[:, jp, i, :, :]  # [128, h, 512] fp8
                    if j in DVE_CHUNKS:
                        nc.vector.tensor_scalar(
                            out=dst.bitcast(u8), in0=st,
                            scalar1=A_HACK, scalar2=B_HACK,
                            op0=mult, op1=add,
                        )
                    else:
                        nc.scalar.activation(
                            out=dst, in_=st, func=Exp, scale=0.125, bias=bias_exp
                        )

                def emit_pv(qg, h, pts, warm=False):
                    pu = ps_u.tile([VW, 512], fp32, tag="u", name=f"u{qg}_{h}")
                    for jp in range(JP):
                        nc.tensor.matmul(
                            pu,
                            Vp[:, jp, :, h * VW : (h + 1) * VW],
                            pts[:, jp, :, h, :],
                            start=(jp == 0), stop=(jp == JP - 1),
                            perf_mode=DR,
                        )
                        if warm and jp == 3:
                            emit_warm(1)
                    pu_sb = usb_pool.tile([VW, 512], fp16, tag="usb")
                    nc.vector.tensor_copy(out=pu_sb, in_=pu)
                    hs = h * DK
                    for s in range(4):
                        ta = ps_w.tile([128, 128], fp16, tag="tw")
                        nc.tensor.transpose(
                            ta[:, 0:VW],
                            pu_sb[:, s * 128 : (s + 1) * 128],
                            ident16[0:VW, 0:VW],
                        )
                        rec = small_pool.tile([128, 1], fp32, tag="rec")
                        nc.vector.reciprocal(rec, ta[:, DK : DK + 1])
                        nc.vector.tensor_scalar_mul(
                            att_sb[:, qg * 4 + s, hs : hs + DK],
                            in0=ta[:, 0:DK],
                            scalar1=rec,
                        )

                # ---- Phase A: loads, projections, qg0 scores ----
                emit_tgt_group(0)
                pts_tiles = {}
                pts_tiles[0] = pt_pool.tile(
                    [128, JP, 2, HPC, 512], fp8, tag="pts", name="pts0"
                )
                for g in range(G):
                    emit_mem_group(g)
                    for j in range(4 * g, 4 * g + 4):
                        emit_score_chunk(0, j, pts_tiles[0])

                for b in range(B):
                    nc.sync.dma_start(
                        out=tgtc_sb[:, b, :, :],
                        in_=tgtc[b, :, :].rearrange("(t p) c -> p t c", p=128),
                    )

                # ---- Phase B: per q-group scores/exp -> PV -> finalize ----
                for qg in range(QG):
                    if qg + 1 < QG:
                        emit_tgt_group(qg + 1)
                        pts_tiles[qg + 1] = pt_pool.tile(
                            [128, JP, 2, HPC, 512], fp8, tag="pts",
                            name=f"pts{qg + 1}",
                        )
                        for j in range(KC):
                            emit_score_chunk(
                                qg + 1, j, pts_tiles[qg + 1], warm=(j % 4 == 2)
                            )
                    for h in range(HPC):
                        emit_pv(qg, h, pts_tiles[qg], warm=(qg == QG - 1))
                    qsl = slice(qg * 512, (qg + 1) * 512)
                    for b in range(B):
                        eng = nc.gpsimd if b < 2 else nc.vector
                        eng.tensor_add(
                            out=tgtc_sb[:, b, qg * 4 : (qg + 1) * 4, :],
                            in0=tgtc_sb[:, b, qg * 4 : (qg + 1) * 4, :],
                            in1=att_sb[:, qg * 4 : (qg + 1) * 4, :],
                        )
                    for b in range(B):
                        nc.sync.dma_start(
                            out=outc[b, qsl, :].rearrange(
                                "(t p) c -> p t c", p=128
                            ),
                            in_=tgtc_sb[:, b, qg * 4 : (qg + 1) * 4, :],
                        )

    nc.finalize()
    return nc


def _get_nc(n_rows):
    if n_rows not in _CACHE:
        _CACHE[n_rows] = _build(n_rows)
    return _CACHE[n_rows]


def _run(tgt, memory, Wq, Wk, Wv, trace=False):
    global LAST_RESULTS
    from concourse.bass_utils import run_bass_kernel_spmd

    n_rows = tgt.shape[1]
    nc = _get_nc(n_rows)

    tgt = np.ascontiguousarray(tgt, dtype=np.float32)
    memory = np.ascontiguousarray(memory, dtype=np.float32)
    tgt0t = np.ascontiguousarray(tgt[0].T).astype(np.float16)
    mem0t = np.ascontiguousarray(memory[0].T).astype(np.float16)

    in_maps = []
    for c in range(NCORES):
        sl = slice(c * CW, (c + 1) * CW)
        in_maps.append(
            {
                "tgt0t": tgt0t,
                "mem0t": mem0t,
                "wqt": np.ascontiguousarray(Wq[sl, :].T).astype(np.float16),
                "wkt": np.ascontiguousarray(Wk[sl, :].T).astype(np.float16),
                "wvt": np.ascontiguousarray(Wv[sl, :].T).astype(np.float16),
                "tgtc": np.ascontiguousarray(tgt[:, :, sl]).astype(np.float16),
            }
        )
    res = run_bass_kernel_spmd(nc, in_maps, list(range(NCORES)), trace=trace)
    LAST_RESULTS = res
    out = np.concatenate(
        [res.results[c]["outc"].astype(np.float32) for c in range(NCORES)], axis=2
    )
    return out


def kernel(tgt, memory, Wq, Wk, Wv):
    return _run(tgt, memory, Wq, Wk, Wv)
